# revision 17
# baseline (speedup 1.0000x reference)
"""Trainium2 Bass kernel for nn_CrossSpatialLearning.

Reference math (B=4, C=256, H=W=64, g=4, cpg=64):
  xg = x.reshape(B,g,cpg,H,W)
  a    = w1 @ xg + b1                         (grouped 1x1 conv)
  xc   = concat(mean_w xg, mean_h xg)         (coordinate pooling)
  y    = w2 @ xc + b2 ; x2 = y_h outer y_w    (per-channel outer product)
  c    = conv3x3_grouped(x, w3) + b3
  att  = sigmoid(a^T c)  per (b,g)            ([HW,HW] attention)
  attd = att @ x2^T
  out  = wf @ concat(a, x2, attd) + bf

Sharding: 16 (b,g) pairs over 8 cores -> core k handles b=k//2 and the two
groups 2*(k%2), 2*(k%2)+1, i.e. channel block [128*(k%2), 128*(k%2)+128) of
C=256.  Both groups are stacked on the 128 SBUF partitions; the grouped
matmuls use block-diagonal stationaries (full K=128) or per-pair
tile_position packing (attention).  Each core computes a partial final
projection (its 384 of 768 concat channels); the host sums the two partials
per batch element (the natural unshard for group-parallel sharding).
"""

import numpy as np
import ml_dtypes
from contextlib import ExitStack

import concourse.bass as bass
import concourse.mybir as mybir
import concourse.tile as tile
from concourse import bacc
from concourse import bass_utils
from concourse.bass_interp import get_hw_module
from concourse.masks import make_identity

F32 = mybir.dt.float32
BF16 = mybir.dt.bfloat16
SIG = mybir.ActivationFunctionType.Sigmoid

B, C, H, W = 4, 256, 64, 64
G, CPG = 4, 64
HW = H * W          # 4096
NCORES = 8

_CACHE = {}


# ----------------------------------------------------------------------------
# device kernel body
# ----------------------------------------------------------------------------

def _kernel_body(tc, d):
    nc = tc.nc
    with ExitStack() as ctx:
        P = ctx.enter_context(tc.tile_pool(name="persist", bufs=1))

        # persistent SBUF tensors (both group-pairs stacked on 128 partitions)
        x32 = P.tile([128, HW], F32)
        xbf = P.tile([128, HW], BF16)
        xm1 = P.tile([128, HW], BF16)   # x shifted: value at w is x[w-1], 0 at w=0
        xp1 = P.tile([128, HW], BF16)   # value at w is x[w+1], 0 at w=63
        af = P.tile([128, HW], BF16)    # a = w1@x + b1   (also final-proj rhs)
        cf = P.tile([128, HW], BF16)    # conv3x3 output
        x2f = P.tile([128, HW], BF16)   # outer-product branch
        x2ft = P.tile([128, 32, 128], BF16)  # x2f transposed, 32 chunks of 128 n
        attd = P.tile([128, HW], BF16)  # attention output
        fpre = P.tile([128, 2, 4096], F32)  # final-proj partial (af+x2f thirds)
        ident = P.tile([128, 128], BF16)

        w1s = P.tile([128, 128], BF16)
        w2s = P.tile([128, 128], BF16)
        w3s = P.tile([128, 9, 128], BF16)
        wfs = P.tile([128, 3, 256], BF16)
        b1s = P.tile([128, 1], F32)
        b2s = P.tile([128, 1], F32)
        b3s = P.tile([128, 1], F32)
        bfs = P.tile([128, 2], F32)
        xc32 = P.tile([128, 128], F32)
        xcbf = P.tile([128, 128], BF16)
        ysb = P.tile([128, 128], F32)

        # input DMAs
        nc.sync.dma_start(x32, d["x"].ap())
        nc.sync.dma_start(w1s, d["w1bd"].ap())
        nc.sync.dma_start(w2s, d["w2bd"].ap())
        nc.sync.dma_start(w3s, d["w3bd"].ap().rearrange("t p m -> p t m"))
        nc.sync.dma_start(wfs, d["wft"].ap().rearrange("t p m -> p t m"))
        nc.sync.dma_start(b1s, d["b1c"].ap())
        nc.sync.dma_start(b2s, d["b2c"].ap())
        nc.sync.dma_start(b3s, d["b3c"].ap())
        nc.sync.dma_start(bfs, d["bfc"].ap().rearrange("(t p) o -> p (t o)", p=128))

        make_identity(nc, ident)

        # PE warmup: ~30 dense back-to-back matmuls (~7us) with no consumers,
        # so the HAM activity monitor lifts the PE clock gate to 2.4 GHz
        # before the real matmul stream starts.
        warm_in = P.tile([128, 512], BF16)
        nc.vector.memset(warm_in, 0.0)

        # cast + shifted copies for the 3x3 conv
        x32v = x32.rearrange("p (h w) -> p h w", w=W)
        xbfv = xbf.rearrange("p (h w) -> p h w", w=W)
        xm1v = xm1.rearrange("p (h w) -> p h w", w=W)
        xp1v = xp1.rearrange("p (h w) -> p h w", w=W)
        nc.vector.tensor_copy(xbf, x32)
        nc.gpsimd.memset(xm1v[:, :, 0:1], 0.0)
        nc.vector.tensor_copy(xm1v[:, :, 1:W], xbfv[:, :, 0 : W - 1])
        nc.gpsimd.memset(xp1v[:, :, W - 1 : W], 0.0)
        nc.vector.tensor_copy(xp1v[:, :, 0 : W - 1], xbfv[:, :, 1:W])

        # coordinate pooling (sums; the 1/64 is folded into w2bd on host)
        nc.vector.tensor_reduce(
            xc32[:, 0:H], x32v, axis=mybir.AxisListType.X, op=mybir.AluOpType.add
        )
        nc.vector.tensor_reduce(
            xc32[:, H : H + W],
            x32v.rearrange("p h w -> p w h"),
            axis=mybir.AxisListType.X,
            op=mybir.AluOpType.add,
        )
        nc.vector.tensor_copy(xcbf, xc32)

        with tc.tile_pool(name="prep_acc", bufs=2, space="PSUM") as ACC, \
             tc.tile_pool(name="prep_sm", bufs=1, space="PSUM") as SM:
            wps = SM.tile([128, 512], F32, tag="warm", bufs=1)
            for _ in range(16):
                nc.tensor.matmul(wps, lhsT=ident, rhs=warm_in)
            # consume once so nothing dead-code-eliminates the warmup
            nc.vector.tensor_copy(warm_in[:, 0:1], wps[:, 0:1])

            # y = w2bd @ xc + b2 ; h2 = y[:, :64], w2s_row = y[:, 64:]
            yps = SM.tile([128, 128], F32, tag="y", bufs=1)
            nc.tensor.matmul(yps, lhsT=w2s, rhs=xcbf)
            nc.vector.tensor_scalar_add(ysb, yps, b2s)
            h2 = ysb[:, 0:H]
            w2row = ysb[:, H : H + W]

            # x2[c, h, w] = h2[c, h] * w2row[c, w]
            x2fv = x2f.rearrange("p (h w) -> p h w", w=W)
            for h in range(H):
                nc.vector.tensor_scalar_mul(x2fv[:, h, :], w2row, h2[:, h : h + 1])
                if h % 2 == 1:
                    j = h // 2
                    nc.sync.dma_start_transpose(
                        x2ft[:, j, :], x2f[:, 128 * j : 128 * j + 128]
                    )

            # a = w1bd @ x + b1
            for q in range(4):
                aps = ACC.tile([128, 1024], F32, tag="acc")
                for j in range(2):
                    s = 1024 * q + 512 * j
                    nc.tensor.matmul(
                        aps[:, 512 * j : 512 * j + 512], lhsT=w1s, rhs=xbf[:, s : s + 512]
                    )
                nc.scalar.add(af[:, 1024 * q : 1024 * q + 1024], aps, b1s)

            # conv3x3 (grouped) as 9 shifted matmuls; center tap first (start=True)
            taps = []
            for dy in (-1, 0, 1):
                for dx in (-1, 0, 1):
                    taps.append((dy, dx))
            tap_order = [(0, 0)] + [t for t in taps if t != (0, 0)]
            for q in range(4):
                cps = ACC.tile([128, 1024], F32, tag="acc")
                for j in range(2):
                    s = 1024 * q + 512 * j
                    for ti, (dy, dx) in enumerate(tap_order):
                        t_idx = (dy + 1) * 3 + (dx + 1)
                        src = xm1 if dx == -1 else (xp1 if dx == 1 else xbf)
                        lo, hi = s, s + 512
                        if dy == -1:
                            lo = max(lo, W)
                        if dy == 1:
                            hi = min(hi, HW - W)
                        nc.tensor.matmul(
                            cps[:, lo - 1024 * q : hi - 1024 * q],
                            lhsT=w3s[:, t_idx, :],
                            rhs=src[:, lo + W * dy : hi + W * dy],
                            start=(ti == 0),
                            stop=(ti == len(tap_order) - 1),
                            skip_group_check=True,
                        )
                nc.scalar.add(cf[:, 1024 * q : 1024 * q + 1024], cps, b3s)

            # final-projection pre-pass: af and x2f thirds accumulated into
            # SBUF now; the post-attention tail only adds the attd third.
            for oh in range(2):
                for nt in range(8):
                    pps = SM.tile([128, 512], F32, tag="fpre", bufs=2)
                    for t in range(2):
                        nc.tensor.matmul(
                            pps,
                            lhsT=wfs[:, t, 128 * oh : 128 * oh + 128],
                            rhs=[af, x2f][t][:, 512 * nt : 512 * nt + 512],
                            start=(t == 0),
                            stop=(t == 1),
                        )
                    nc.vector.tensor_scalar_add(
                        fpre[:, oh, 512 * nt : 512 * nt + 512], pps, bfs[:, oh : oh + 1]
                    )


        # ------------------------------------------------------------------
        # attention: att = sigmoid(af^T cf); attd = att @ x2f^T   per pair
        # pair0 on partitions 0:64, pair1 on 64:128 (tile_position packing)
        # ------------------------------------------------------------------
        # In the first quarter, the m2 (attd) matmuls lag the m1/sigmoid
        # stream by LAG chunks so sigmoids start as soon as af/cf are ready
        # while the DMA-engine transposes of x2f are still completing.
        LAG = 12
        # Every OFF_EVERY-th chunk, the pair-1 sigmoid is computed as a
        # clamped odd polynomial on an otherwise idle engine (alternating
        # VectorE / GpSimdE) instead of ScalarE, which is the saturated
        # engine.  deg-5 in z^2 on [-8, 8], max abs err 8.7e-3.
        OFF_EVERY = 8
        PC = [2.4030832888e-01, -1.4481238503e-02, 6.4256318472e-04,
              -1.6011602528e-05, 2.0174960267e-07, -9.9789517200e-10]
        with tc.tile_pool(name="att_lg", bufs=2, space="PSUM") as LG, \
             tc.tile_pool(name="att_acc", bufs=2, space="PSUM") as DA, \
             tc.tile_pool(name="att_sb", bufs=LAG + 2) as SB, \
             tc.tile_pool(name="att_poly", bufs=1) as OF:

            def emit_poly_sigmoid(lg1, s1, use_gpsimd):
                e = nc.gpsimd if use_gpsimd else nc.vector
                zc = OF.tile([128, 1024], F32, tag="zc")
                u = OF.tile([128, 1024], F32, tag="u")
                pp = OF.tile([128, 1024], F32, tag="pp")
                # clamp must run on VectorE (GpSimd has no PSUM port)
                nc.vector.tensor_scalar(
                    zc, lg1, 8.0, -8.0, mybir.AluOpType.min, mybir.AluOpType.max
                )
                e.tensor_tensor(u, zc, zc, mybir.AluOpType.mult)
                e.tensor_scalar(pp, u, PC[5], PC[4], mybir.AluOpType.mult,
                                mybir.AluOpType.add)
                for ck in (PC[3], PC[2], PC[1], PC[0]):
                    e.tensor_tensor(pp, pp, u, mybir.AluOpType.mult)
                    e.tensor_scalar_add(pp, pp, ck)
                e.tensor_tensor(s1, zc, pp, mybir.AluOpType.mult)
                e.tensor_scalar_add(s1, s1, 0.5)

            def emit_m1(mq, ch):
                gi = 32 * mq + ch
                lg0 = LG.tile([128, 1024], F32, tag="lg", name="lg0")
                lg1 = LG.tile([128, 1024], F32, tag="lg", name="lg1")
                cs = 128 * ch
                for p, lg in ((0, lg0), (1, lg1)):
                    pl = slice(64 * p, 64 * p + 64)
                    for j in range(2):
                        m0 = 1024 * mq + 512 * j
                        nc.tensor.matmul(
                            lg[:, 512 * j : 512 * j + 512],
                            lhsT=cf[pl, cs : cs + 128],
                            rhs=af[pl, m0 : m0 + 512],
                        )
                s0 = SB.tile([128, 1024], BF16, tag="sg", name="s0")
                s1 = SB.tile([128, 1024], BF16, tag="sg", name="s1")
                nc.scalar.activation(s0, lg0, SIG)
                if gi % OFF_EVERY == OFF_EVERY // 2 and gi > LAG:
                    emit_poly_sigmoid(lg1, s1, use_gpsimd=(gi // OFF_EVERY) % 2 == 0)
                else:
                    nc.scalar.activation(s1, lg1, SIG)
                return s0, s1

            def emit_m2(dps, ch, s0, s1):
                st, sp = (ch == 0), (ch == 31)
                for p, s in ((0, s0), (1, s1)):
                    for j in range(2):
                        nc.tensor.matmul(
                            dps[64 * p : 64 * p + 64, 512 * j : 512 * j + 512],
                            lhsT=x2ft[:, ch, 64 * p : 64 * p + 64],
                            rhs=s[:, 512 * j : 512 * j + 512],
                            start=st,
                            stop=sp,
                            skip_group_check=True,
                        )

            for mq in range(4):
                dps = DA.tile([128, 1024], F32, tag="attd")
                lag = LAG if mq == 0 else 8
                pend = {}
                for step in range(32 + lag):
                    if step < 32:
                        pend[step] = emit_m1(mq, step)
                    if step >= lag:
                        emit_m2(dps, step - lag, *pend.pop(step - lag))
                nc.vector.tensor_copy(attd[:, 1024 * mq : 1024 * mq + 1024], dps)

        # ------------------------------------------------------------------
        # final projection tail: attd third + pre-pass + bias
        # ------------------------------------------------------------------
        out_ap = d["out"].ap()
        with tc.tile_pool(name="fin_ps", bufs=4, space="PSUM") as FP, \
             tc.tile_pool(name="fin_sb", bufs=4) as FO:
            for oh in range(2):
                for nt in range(8):
                    ps = FP.tile([128, 512], F32, tag="fp")
                    nc.tensor.matmul(
                        ps,
                        lhsT=wfs[:, 2, 128 * oh : 128 * oh + 128],
                        rhs=attd[:, 512 * nt : 512 * nt + 512],
                    )
                    ot = FO.tile([128, 512], F32, tag="ot")
                    nc.vector.tensor_add(
                        ot, ps, fpre[:, oh, 512 * nt : 512 * nt + 512]
                    )
                    nc.sync.dma_start(
                        out_ap[128 * oh : 128 * oh + 128, 512 * nt : 512 * nt + 512], ot
                    )


def _tune_compiler_flags():
    """Re-enable walrus's LDWEIGHTS dedup (the container default disables it).
    Redundant per-matmul weight reloads cost ~140us on the PE stream here."""
    try:
        from concourse.compiler_utils import get_compiler_flags, set_compiler_flags

        flags = get_compiler_flags()
        new = [
            f.replace("--enable-ldw-opt=false", "--enable-ldw-opt=true")
            for f in flags
        ]
        if new != flags:
            set_compiler_flags(new)
    except Exception as e:
        print(f"compiler flag tune skipped: {e}")


def _build():
    if "nc" in _CACHE:
        return _CACHE["nc"]
    _tune_compiler_flags()
    nc = bacc.Bacc(
        "TRN2",
        target_bir_lowering=False,
        debug=False,
        enable_asserts=False,
        num_devices=NCORES,
    )
    d = {
        "x": nc.dram_tensor("x", [128, HW], F32, kind="ExternalInput"),
        "w1bd": nc.dram_tensor("w1bd", [128, 128], BF16, kind="ExternalInput"),
        "w2bd": nc.dram_tensor("w2bd", [128, 128], BF16, kind="ExternalInput"),
        "w3bd": nc.dram_tensor("w3bd", [9, 128, 128], BF16, kind="ExternalInput"),
        "wft": nc.dram_tensor("wft", [3, 128, 256], BF16, kind="ExternalInput"),
        "b1c": nc.dram_tensor("b1c", [128, 1], F32, kind="ExternalInput"),
        "b2c": nc.dram_tensor("b2c", [128, 1], F32, kind="ExternalInput"),
        "b3c": nc.dram_tensor("b3c", [128, 1], F32, kind="ExternalInput"),
        "bfc": nc.dram_tensor("bfc", [256, 1], F32, kind="ExternalInput"),
        "out": nc.dram_tensor("out", [256, HW], F32, kind="ExternalOutput"),
    }
    with tile.TileContext(nc) as tc:
        _kernel_body(tc, d)
    nc.compile()
    nc.m = get_hw_module(nc.m)
    _CACHE["nc"] = nc
    return nc


# ----------------------------------------------------------------------------
# host side: shard, run, unshard
# ----------------------------------------------------------------------------

def _bf(a):
    return np.asarray(a, np.float32).astype(ml_dtypes.bfloat16)


def _shard_inputs(x, w1, b1, w2, b2, w3, b3, wf, bf):
    x = np.asarray(x, np.float32).reshape(B, C, HW)
    w1 = np.asarray(w1, np.float32)
    w2 = np.asarray(w2, np.float32)
    w3 = np.asarray(w3, np.float32)
    wf = np.asarray(wf, np.float32)
    b1 = np.asarray(b1, np.float32)
    b2 = np.asarray(b2, np.float32)
    b3 = np.asarray(b3, np.float32)
    bf = np.asarray(bf, np.float32)

    in_maps = []
    for k in range(NCORES):
        b = k // 2
        half = k % 2
        cs = slice(128 * half, 128 * half + 128)
        g0 = 2 * half

        w1bd = np.zeros((128, 128), np.float32)
        w1bd[0:64, 0:64] = w1[g0].T
        w1bd[64:128, 64:128] = w1[g0 + 1].T

        w2bd = np.zeros((128, 128), np.float32)
        w2bd[0:64, 0:64] = (w2[g0] / 64.0).T
        w2bd[64:128, 64:128] = (w2[g0 + 1] / 64.0).T

        w3bd = np.zeros((9, 128, 128), np.float32)
        for dy in (-1, 0, 1):
            for dx in (-1, 0, 1):
                t = (dy + 1) * 3 + (dx + 1)
                blk = w3[cs, :, dy + 1, dx + 1]  # [128 o, 64 i]
                w3bd[t, 0:64, 0:64] = blk[0:64].T
                w3bd[t, 64:128, 64:128] = blk[64:128].T

        # concat in the reference is per-group along cpg: flat channel of the
        # 768-wide concat is g*192 + t*64 + c  (t in {a, x2, attd})
        wft = np.stack(
            [
                np.concatenate(
                    [
                        wf[:, 192 * g0 + 64 * t : 192 * g0 + 64 * t + 64],
                        wf[:, 192 * (g0 + 1) + 64 * t : 192 * (g0 + 1) + 64 * t + 64],
                    ],
                    axis=1,
                ).T
                for t in range(3)
            ]
        )  # [3, 128 cc, 256 o]

        bfc = bf if half == 0 else np.zeros_like(bf)

        in_maps.append(
            {
                "x": np.ascontiguousarray(x[b, cs]),
                "w1bd": _bf(w1bd),
                "w2bd": _bf(w2bd),
                "w3bd": _bf(w3bd),
                "wft": _bf(wft),
                "b1c": np.ascontiguousarray(b1[cs].reshape(128, 1)),
                "b2c": np.ascontiguousarray(b2[cs].reshape(128, 1)),
                "b3c": np.ascontiguousarray(b3[cs].reshape(128, 1)),
                "bfc": np.ascontiguousarray(bfc.reshape(256, 1)),
            }
        )
    return in_maps


def _ensure_ntff_hook():
    """The container's antenv lacks axon_hooks, so boot() silently skips
    registering the NTFF profile hook.  Recreate the module and register the
    ctypes-based hook so trace=True yields exec_time_ns."""
    import sys
    import types

    try:
        import antenv.axon_hooks  # noqa: F401
        return
    except ImportError:
        pass
    try:
        import antenv
        from trn_agent_boot.trn_boot import _ntff_profile_via_ctypes

        mod = types.ModuleType("antenv.axon_hooks")
        holder = {}
        mod.set_axon_ntff_profile_hook = lambda h: holder.__setitem__("h", h)
        mod.get_axon_ntff_profile_hook = lambda: holder.get("h")
        sys.modules["antenv.axon_hooks"] = mod
        antenv.axon_hooks = mod
        mod.set_axon_ntff_profile_hook(
            _ntff_profile_via_ctypes("/opt/axon/libaxon_pjrt.so")
        )
        # artifact upload needs S3 creds we don't have; make it a no-op
        bass_utils.upload_artifacts = lambda tmpdir: tmpdir
    except Exception as e:  # profiling is best-effort
        print(f"ntff hook setup failed: {e}")


def run(inputs, trace=False):
    if trace:
        _ensure_ntff_hook()
    nc = _build()
    in_maps = _shard_inputs(**inputs)
    res = bass_utils.run_bass_kernel_spmd(
        nc, in_maps, core_ids=list(range(NCORES)), trace=trace
    )
    parts = [res.results[k]["out"].astype(np.float32) for k in range(NCORES)]
    out = np.empty((B, C, HW), np.float32)
    for b in range(B):
        out[b] = parts[2 * b] + parts[2 * b + 1]
    return out.reshape(B, C, H, W), res


def kernel(**inputs) -> np.ndarray:
    out, _ = run(inputs, trace=False)
    return out


# revision 18
# speedup vs baseline: 3.2468x; 3.2468x over previous
"""Trainium2 Bass kernel for nn_CrossSpatialLearning.

Reference math (B=4, C=256, H=W=64, g=4, cpg=64):
  xg = x.reshape(B,g,cpg,H,W)
  a    = w1 @ xg + b1                         (grouped 1x1 conv)
  xc   = concat(mean_w xg, mean_h xg)         (coordinate pooling)
  y    = w2 @ xc + b2 ; x2 = y_h outer y_w    (per-channel outer product)
  c    = conv3x3_grouped(x, w3) + b3
  att  = sigmoid(a^T c)  per (b,g)            ([HW,HW] attention)
  attd = att @ x2^T
  out  = wf @ concat(a, x2, attd) + bf

Sharding: 16 (b,g) pairs over 8 cores -> core k handles b=k//2 and the two
groups 2*(k%2), 2*(k%2)+1, i.e. channel block [128*(k%2), 128*(k%2)+128) of
C=256.  Both groups are stacked on the 128 SBUF partitions; the grouped
matmuls use block-diagonal stationaries (full K=128) or per-pair
tile_position packing (attention).  Each core computes a partial final
projection (its 384 of 768 concat channels); the host sums the two partials
per batch element (the natural unshard for group-parallel sharding).
"""

import numpy as np
import ml_dtypes
from contextlib import ExitStack

import concourse.bass as bass
import concourse.mybir as mybir
import concourse.tile as tile
from concourse import bacc
from concourse import bass_utils
from concourse.bass_interp import get_hw_module
from concourse.masks import make_identity

F32 = mybir.dt.float32
BF16 = mybir.dt.bfloat16
SIG = mybir.ActivationFunctionType.Sigmoid

B, C, H, W = 4, 256, 64, 64
G, CPG = 4, 64
HW = H * W          # 4096
NCORES = 8

_CACHE = {}


# ----------------------------------------------------------------------------
# device kernel body
# ----------------------------------------------------------------------------

def _kernel_body(tc, d):
    nc = tc.nc
    with ExitStack() as ctx:
        P = ctx.enter_context(tc.tile_pool(name="persist", bufs=1))

        # persistent SBUF tensors (both group-pairs stacked on 128 partitions)
        x32 = P.tile([128, HW], F32)
        xbf = P.tile([128, HW], BF16)
        xm1 = P.tile([128, HW], BF16)   # x shifted: value at w is x[w-1], 0 at w=0
        xp1 = P.tile([128, HW], BF16)   # value at w is x[w+1], 0 at w=63
        af = P.tile([128, HW], BF16)    # a = w1@x + b1   (also final-proj rhs)
        cf = P.tile([128, HW], BF16)    # conv3x3 output
        x2f = P.tile([128, HW], BF16)   # outer-product branch
        x2ft = P.tile([128, 32, 128], BF16)  # x2f transposed, 32 chunks of 128 n
        attd = P.tile([128, HW], BF16)  # attention output
        fpre = P.tile([128, 2, 4096], F32)  # final-proj partial (af+x2f thirds)
        ident = P.tile([128, 128], BF16)

        w1s = P.tile([128, 128], BF16)
        w2s = P.tile([128, 128], BF16)
        w3s = P.tile([128, 9, 128], BF16)
        wfs = P.tile([128, 3, 256], BF16)
        b1s = P.tile([128, 1], F32)
        b2s = P.tile([128, 1], F32)
        b3s = P.tile([128, 1], F32)
        bfs = P.tile([128, 2], F32)
        xc32 = P.tile([128, 128], F32)
        xcbf = P.tile([128, 128], BF16)
        ysb = P.tile([128, 128], F32)

        # input DMAs
        nc.sync.dma_start(x32, d["x"].ap())
        nc.sync.dma_start(w1s, d["w1bd"].ap())
        nc.sync.dma_start(w2s, d["w2bd"].ap())
        nc.sync.dma_start(w3s, d["w3bd"].ap().rearrange("t p m -> p t m"))
        nc.sync.dma_start(wfs, d["wft"].ap().rearrange("t p m -> p t m"))
        nc.sync.dma_start(b1s, d["b1c"].ap())
        nc.sync.dma_start(b2s, d["b2c"].ap())
        nc.sync.dma_start(b3s, d["b3c"].ap())
        nc.sync.dma_start(bfs, d["bfc"].ap().rearrange("(t p) o -> p (t o)", p=128))

        make_identity(nc, ident)

        # PE warmup: ~30 dense back-to-back matmuls (~7us) with no consumers,
        # so the HAM activity monitor lifts the PE clock gate to 2.4 GHz
        # before the real matmul stream starts.
        warm_in = P.tile([128, 512], BF16)
        nc.vector.memset(warm_in, 0.0)

        # cast + shifted copies for the 3x3 conv
        x32v = x32.rearrange("p (h w) -> p h w", w=W)
        xbfv = xbf.rearrange("p (h w) -> p h w", w=W)
        xm1v = xm1.rearrange("p (h w) -> p h w", w=W)
        xp1v = xp1.rearrange("p (h w) -> p h w", w=W)
        nc.vector.tensor_copy(xbf, x32)
        nc.gpsimd.memset(xm1v[:, :, 0:1], 0.0)
        nc.vector.tensor_copy(xm1v[:, :, 1:W], xbfv[:, :, 0 : W - 1])
        nc.gpsimd.memset(xp1v[:, :, W - 1 : W], 0.0)
        nc.vector.tensor_copy(xp1v[:, :, 0 : W - 1], xbfv[:, :, 1:W])

        # coordinate pooling (sums; the 1/64 is folded into w2bd on host)
        nc.vector.tensor_reduce(
            xc32[:, 0:H], x32v, axis=mybir.AxisListType.X, op=mybir.AluOpType.add
        )
        nc.vector.tensor_reduce(
            xc32[:, H : H + W],
            x32v.rearrange("p h w -> p w h"),
            axis=mybir.AxisListType.X,
            op=mybir.AluOpType.add,
        )
        nc.vector.tensor_copy(xcbf, xc32)

        with tc.tile_pool(name="prep_acc", bufs=2, space="PSUM") as ACC, \
             tc.tile_pool(name="prep_sm", bufs=1, space="PSUM") as SM:
            wps = SM.tile([128, 512], F32, tag="warm", bufs=1)
            for _ in range(16):
                nc.tensor.matmul(wps, lhsT=ident, rhs=warm_in)
            # consume once so nothing dead-code-eliminates the warmup
            nc.vector.tensor_copy(warm_in[:, 0:1], wps[:, 0:1])

            # y = w2bd @ xc + b2 ; h2 = y[:, :64], w2s_row = y[:, 64:]
            yps = SM.tile([128, 128], F32, tag="y", bufs=1)
            nc.tensor.matmul(yps, lhsT=w2s, rhs=xcbf)
            nc.vector.tensor_scalar_add(ysb, yps, b2s)
            h2 = ysb[:, 0:H]
            w2row = ysb[:, H : H + W]

            # x2[c, h, w] = h2[c, h] * w2row[c, w]
            x2fv = x2f.rearrange("p (h w) -> p h w", w=W)
            for h in range(H):
                nc.vector.tensor_scalar_mul(x2fv[:, h, :], w2row, h2[:, h : h + 1])
                if h % 2 == 1:
                    j = h // 2
                    nc.sync.dma_start_transpose(
                        x2ft[:, j, :], x2f[:, 128 * j : 128 * j + 128]
                    )

            # a = w1bd @ x + b1
            for q in range(4):
                aps = ACC.tile([128, 1024], F32, tag="acc")
                for j in range(2):
                    s = 1024 * q + 512 * j
                    nc.tensor.matmul(
                        aps[:, 512 * j : 512 * j + 512], lhsT=w1s, rhs=xbf[:, s : s + 512]
                    )
                nc.scalar.add(af[:, 1024 * q : 1024 * q + 1024], aps, b1s)

            # conv3x3 (grouped) as 9 shifted matmuls; center tap first (start=True)
            taps = []
            for dy in (-1, 0, 1):
                for dx in (-1, 0, 1):
                    taps.append((dy, dx))
            tap_order = [(0, 0)] + [t for t in taps if t != (0, 0)]
            for q in range(4):
                cps = ACC.tile([128, 1024], F32, tag="acc")
                for j in range(2):
                    s = 1024 * q + 512 * j
                    for ti, (dy, dx) in enumerate(tap_order):
                        t_idx = (dy + 1) * 3 + (dx + 1)
                        src = xm1 if dx == -1 else (xp1 if dx == 1 else xbf)
                        lo, hi = s, s + 512
                        if dy == -1:
                            lo = max(lo, W)
                        if dy == 1:
                            hi = min(hi, HW - W)
                        nc.tensor.matmul(
                            cps[:, lo - 1024 * q : hi - 1024 * q],
                            lhsT=w3s[:, t_idx, :],
                            rhs=src[:, lo + W * dy : hi + W * dy],
                            start=(ti == 0),
                            stop=(ti == len(tap_order) - 1),
                            skip_group_check=True,
                        )
                nc.scalar.add(cf[:, 1024 * q : 1024 * q + 1024], cps, b3s)

            # final-projection pre-pass: af and x2f thirds accumulated into
            # SBUF now; the post-attention tail only adds the attd third.
            for oh in range(2):
                for nt in range(8):
                    pps = SM.tile([128, 512], F32, tag="fpre", bufs=2)
                    for t in range(2):
                        nc.tensor.matmul(
                            pps,
                            lhsT=wfs[:, t, 128 * oh : 128 * oh + 128],
                            rhs=[af, x2f][t][:, 512 * nt : 512 * nt + 512],
                            start=(t == 0),
                            stop=(t == 1),
                        )
                    nc.vector.tensor_scalar_add(
                        fpre[:, oh, 512 * nt : 512 * nt + 512], pps, bfs[:, oh : oh + 1]
                    )


        # ------------------------------------------------------------------
        # attention: att = sigmoid(af^T cf); attd = att @ x2f^T   per pair
        # pair0 on partitions 0:64, pair1 on 64:128 (tile_position packing)
        # ------------------------------------------------------------------
        # In the first quarter, the m2 (attd) matmuls lag the m1/sigmoid
        # stream by LAG chunks so sigmoids start as soon as af/cf are ready
        # while the DMA-engine transposes of x2f are still completing.
        LAG = 12
        # Every OFF_EVERY-th chunk, the pair-1 sigmoid is computed as a
        # clamped odd polynomial on an otherwise idle engine (alternating
        # VectorE / GpSimdE) instead of ScalarE, which is the saturated
        # engine.  deg-5 in z^2 on [-8, 8], max abs err 8.7e-3.
        OFF_EVERY = 8
        PC = [2.4030832888e-01, -1.4481238503e-02, 6.4256318472e-04,
              -1.6011602528e-05, 2.0174960267e-07, -9.9789517200e-10]
        with tc.tile_pool(name="att_lg", bufs=2, space="PSUM") as LG, \
             tc.tile_pool(name="att_acc", bufs=2, space="PSUM") as DA, \
             tc.tile_pool(name="att_sb", bufs=LAG + 2) as SB, \
             tc.tile_pool(name="att_poly", bufs=1) as OF:

            def emit_poly_sigmoid(lg1, s1, use_gpsimd):
                e = nc.gpsimd if use_gpsimd else nc.vector
                zc = OF.tile([128, 1024], F32, tag="zc")
                u = OF.tile([128, 1024], F32, tag="u")
                pp = OF.tile([128, 1024], F32, tag="pp")
                # clamp must run on VectorE (GpSimd has no PSUM port)
                nc.vector.tensor_scalar(
                    zc, lg1, 8.0, -8.0, mybir.AluOpType.min, mybir.AluOpType.max
                )
                e.tensor_tensor(u, zc, zc, mybir.AluOpType.mult)
                e.tensor_scalar(pp, u, PC[5], PC[4], mybir.AluOpType.mult,
                                mybir.AluOpType.add)
                for ck in (PC[3], PC[2], PC[1], PC[0]):
                    e.tensor_tensor(pp, pp, u, mybir.AluOpType.mult)
                    e.tensor_scalar_add(pp, pp, ck)
                e.tensor_tensor(s1, zc, pp, mybir.AluOpType.mult)
                e.tensor_scalar_add(s1, s1, 0.5)

            def emit_m1(mq, ch):
                gi = 32 * mq + ch
                lg0 = LG.tile([128, 1024], F32, tag="lg", name="lg0")
                lg1 = LG.tile([128, 1024], F32, tag="lg", name="lg1")
                cs = 128 * ch
                for p, lg in ((0, lg0), (1, lg1)):
                    pl = slice(64 * p, 64 * p + 64)
                    for j in range(2):
                        m0 = 1024 * mq + 512 * j
                        nc.tensor.matmul(
                            lg[:, 512 * j : 512 * j + 512],
                            lhsT=cf[pl, cs : cs + 128],
                            rhs=af[pl, m0 : m0 + 512],
                        )
                s0 = SB.tile([128, 1024], BF16, tag="sg", name="s0")
                s1 = SB.tile([128, 1024], BF16, tag="sg", name="s1")
                nc.scalar.activation(s0, lg0, SIG)
                nc.scalar.activation(s1, lg1, SIG)
                return s0, s1

            def emit_m2(dps, ch, s0, s1):
                st, sp = (ch == 0), (ch == 31)
                for p, s in ((0, s0), (1, s1)):
                    for j in range(2):
                        nc.tensor.matmul(
                            dps[64 * p : 64 * p + 64, 512 * j : 512 * j + 512],
                            lhsT=x2ft[:, ch, 64 * p : 64 * p + 64],
                            rhs=s[:, 512 * j : 512 * j + 512],
                            start=st,
                            stop=sp,
                            skip_group_check=True,
                        )

            for mq in range(4):
                dps = DA.tile([128, 1024], F32, tag="attd")
                lag = LAG if mq == 0 else 1
                pend = {}
                for step in range(32 + lag):
                    if step < 32:
                        pend[step] = emit_m1(mq, step)
                    if step >= lag:
                        emit_m2(dps, step - lag, *pend.pop(step - lag))
                nc.vector.tensor_copy(attd[:, 1024 * mq : 1024 * mq + 1024], dps)

        # ------------------------------------------------------------------
        # final projection tail: attd third + pre-pass + bias
        # ------------------------------------------------------------------
        out_ap = d["out"].ap()
        with tc.tile_pool(name="fin_ps", bufs=4, space="PSUM") as FP, \
             tc.tile_pool(name="fin_sb", bufs=4) as FO:
            for oh in range(2):
                for nt in range(8):
                    ps = FP.tile([128, 512], F32, tag="fp")
                    nc.tensor.matmul(
                        ps,
                        lhsT=wfs[:, 2, 128 * oh : 128 * oh + 128],
                        rhs=attd[:, 512 * nt : 512 * nt + 512],
                    )
                    ot = FO.tile([128, 512], F32, tag="ot")
                    nc.vector.tensor_add(
                        ot, ps, fpre[:, oh, 512 * nt : 512 * nt + 512]
                    )
                    nc.sync.dma_start(
                        out_ap[128 * oh : 128 * oh + 128, 512 * nt : 512 * nt + 512], ot
                    )


def _tune_compiler_flags():
    """Re-enable walrus's LDWEIGHTS dedup (the container default disables it).
    Redundant per-matmul weight reloads cost ~140us on the PE stream here."""
    try:
        from concourse.compiler_utils import get_compiler_flags, set_compiler_flags

        flags = get_compiler_flags()
        new = [
            f.replace("--enable-ldw-opt=false", "--enable-ldw-opt=true")
            for f in flags
        ]
        if new != flags:
            set_compiler_flags(new)
    except Exception as e:
        print(f"compiler flag tune skipped: {e}")


def _build():
    if "nc" in _CACHE:
        return _CACHE["nc"]
    _tune_compiler_flags()
    nc = bacc.Bacc(
        "TRN2",
        target_bir_lowering=False,
        debug=False,
        enable_asserts=False,
        num_devices=NCORES,
    )
    d = {
        "x": nc.dram_tensor("x", [128, HW], F32, kind="ExternalInput"),
        "w1bd": nc.dram_tensor("w1bd", [128, 128], BF16, kind="ExternalInput"),
        "w2bd": nc.dram_tensor("w2bd", [128, 128], BF16, kind="ExternalInput"),
        "w3bd": nc.dram_tensor("w3bd", [9, 128, 128], BF16, kind="ExternalInput"),
        "wft": nc.dram_tensor("wft", [3, 128, 256], BF16, kind="ExternalInput"),
        "b1c": nc.dram_tensor("b1c", [128, 1], F32, kind="ExternalInput"),
        "b2c": nc.dram_tensor("b2c", [128, 1], F32, kind="ExternalInput"),
        "b3c": nc.dram_tensor("b3c", [128, 1], F32, kind="ExternalInput"),
        "bfc": nc.dram_tensor("bfc", [256, 1], F32, kind="ExternalInput"),
        "out": nc.dram_tensor("out", [256, HW], F32, kind="ExternalOutput"),
    }
    with tile.TileContext(nc) as tc:
        _kernel_body(tc, d)
    nc.compile()
    nc.m = get_hw_module(nc.m)
    _CACHE["nc"] = nc
    return nc


# ----------------------------------------------------------------------------
# host side: shard, run, unshard
# ----------------------------------------------------------------------------

def _bf(a):
    return np.asarray(a, np.float32).astype(ml_dtypes.bfloat16)


def _shard_inputs(x, w1, b1, w2, b2, w3, b3, wf, bf):
    x = np.asarray(x, np.float32).reshape(B, C, HW)
    w1 = np.asarray(w1, np.float32)
    w2 = np.asarray(w2, np.float32)
    w3 = np.asarray(w3, np.float32)
    wf = np.asarray(wf, np.float32)
    b1 = np.asarray(b1, np.float32)
    b2 = np.asarray(b2, np.float32)
    b3 = np.asarray(b3, np.float32)
    bf = np.asarray(bf, np.float32)

    in_maps = []
    for k in range(NCORES):
        b = k // 2
        half = k % 2
        cs = slice(128 * half, 128 * half + 128)
        g0 = 2 * half

        w1bd = np.zeros((128, 128), np.float32)
        w1bd[0:64, 0:64] = w1[g0].T
        w1bd[64:128, 64:128] = w1[g0 + 1].T

        w2bd = np.zeros((128, 128), np.float32)
        w2bd[0:64, 0:64] = (w2[g0] / 64.0).T
        w2bd[64:128, 64:128] = (w2[g0 + 1] / 64.0).T

        w3bd = np.zeros((9, 128, 128), np.float32)
        for dy in (-1, 0, 1):
            for dx in (-1, 0, 1):
                t = (dy + 1) * 3 + (dx + 1)
                blk = w3[cs, :, dy + 1, dx + 1]  # [128 o, 64 i]
                w3bd[t, 0:64, 0:64] = blk[0:64].T
                w3bd[t, 64:128, 64:128] = blk[64:128].T

        # concat in the reference is per-group along cpg: flat channel of the
        # 768-wide concat is g*192 + t*64 + c  (t in {a, x2, attd})
        wft = np.stack(
            [
                np.concatenate(
                    [
                        wf[:, 192 * g0 + 64 * t : 192 * g0 + 64 * t + 64],
                        wf[:, 192 * (g0 + 1) + 64 * t : 192 * (g0 + 1) + 64 * t + 64],
                    ],
                    axis=1,
                ).T
                for t in range(3)
            ]
        )  # [3, 128 cc, 256 o]

        bfc = bf if half == 0 else np.zeros_like(bf)

        in_maps.append(
            {
                "x": np.ascontiguousarray(x[b, cs]),
                "w1bd": _bf(w1bd),
                "w2bd": _bf(w2bd),
                "w3bd": _bf(w3bd),
                "wft": _bf(wft),
                "b1c": np.ascontiguousarray(b1[cs].reshape(128, 1)),
                "b2c": np.ascontiguousarray(b2[cs].reshape(128, 1)),
                "b3c": np.ascontiguousarray(b3[cs].reshape(128, 1)),
                "bfc": np.ascontiguousarray(bfc.reshape(256, 1)),
            }
        )
    return in_maps


def _ensure_ntff_hook():
    """The container's antenv lacks axon_hooks, so boot() silently skips
    registering the NTFF profile hook.  Recreate the module and register the
    ctypes-based hook so trace=True yields exec_time_ns."""
    import sys
    import types

    try:
        import antenv.axon_hooks  # noqa: F401
        return
    except ImportError:
        pass
    try:
        import antenv
        from trn_agent_boot.trn_boot import _ntff_profile_via_ctypes

        mod = types.ModuleType("antenv.axon_hooks")
        holder = {}
        mod.set_axon_ntff_profile_hook = lambda h: holder.__setitem__("h", h)
        mod.get_axon_ntff_profile_hook = lambda: holder.get("h")
        sys.modules["antenv.axon_hooks"] = mod
        antenv.axon_hooks = mod
        mod.set_axon_ntff_profile_hook(
            _ntff_profile_via_ctypes("/opt/axon/libaxon_pjrt.so")
        )
        # artifact upload needs S3 creds we don't have; make it a no-op
        bass_utils.upload_artifacts = lambda tmpdir: tmpdir
    except Exception as e:  # profiling is best-effort
        print(f"ntff hook setup failed: {e}")


def run(inputs, trace=False):
    if trace:
        _ensure_ntff_hook()
    nc = _build()
    in_maps = _shard_inputs(**inputs)
    res = bass_utils.run_bass_kernel_spmd(
        nc, in_maps, core_ids=list(range(NCORES)), trace=trace
    )
    parts = [res.results[k]["out"].astype(np.float32) for k in range(NCORES)]
    out = np.empty((B, C, HW), np.float32)
    for b in range(B):
        out[b] = parts[2 * b] + parts[2 * b + 1]
    return out.reshape(B, C, H, W), res


def kernel(**inputs) -> np.ndarray:
    out, _ = run(inputs, trace=False)
    return out
